# revision 3
# baseline (speedup 1.0000x reference)
"""Trainium2 distributed kernel for AnatomicalConsistencyLoss, v3.

Like v2 (see kernel_v2.py docstring for the engine-assignment rationale:
VectorE does all tensor_tensor work in the 2x bf16 mode, ScalarE all
unaries + accum reductions, other engines measured counterproductive),
plus:

- The w-stage (S_w / D_w) runs ONCE over the full 22-plane input volume
  instead of per-slab (saves the 2-plane slab halo reprocessing), tiled
  by DMA chunk for overlap with the input transfer.
- sw/dw and everything downstream are stored with dense 20-wide rows,
  so all h/d-stage and pointwise ops are flat 1-D contiguous slices
  (junk h-rows remain, zeroed/one'd in the field tiles and exactly
  subtracted host-side).
- The input tile and u-scratch live in a pool closed after the w-stage
  so the slab-phase tiles reuse their SBUF.
"""

import sys

import numpy as np

sys.path.insert(0, "/opt/trn_rl_repo")

import ml_dtypes

N_CORES = 8
BC = 20
BB = 22
PL = BB * BB     # 484
FB = BB ** 3     # 10648
NVOX = 2 * 160 * 160 * 160
WEIGHT = 0.2

SD = 5
NSLAB = 4
HF = SD * BC * BB    # 2200: dense field half size
NR = SD * BB         # 110 rows per half
JUNK_PER_SLAB = 10 * BC

# DMA/w-stage chunks in planes
CHUNKS = [(0, 3), (3, 8), (8, 13), (13, 17), (17, 22)]

_cache = {}


def _build():
    import concourse.bacc as bacc
    import concourse.tile as tile
    from concourse import mybir

    f32 = mybir.dt.float32
    bf16 = mybir.dt.bfloat16
    AF = mybir.ActivationFunctionType

    nc = bacc.Bacc(
        "TRN2",
        target_bir_lowering=False,
        debug=False,
        enable_asserts=False,
        num_devices=N_CORES,
    )
    xx_d = nc.dram_tensor("xx", [128, 2 * FB], bf16, kind="ExternalInput")
    out_d = nc.dram_tensor("out", [128, 48], f32, kind="ExternalOutput")

    with tile.TileContext(nc) as tc:
        with tc.tile_pool(name="pers", bufs=1) as pers:
            acc = pers.tile([128, 48], f32, tag="acc")
            sw0 = pers.tile([128, 9680], bf16, tag="sw0")
            sw1 = pers.tile([128, 9680], bf16, tag="sw1")
            dw0 = pers.tile([128, 9680], bf16, tag="dw0")
            dw1 = pers.tile([128, 9680], bf16, tag="dw1")
            swf = [sw0, sw1]
            dwf = [dw0, dw1]

            # ---- w-stage over the full volume, chunked by DMA arrival
            with tc.tile_pool(name="xp", bufs=1) as xp:
                xw = xp.tile([128, 2 * FB], bf16, tag="xw")
                U = xp.tile([128, 132 * 21], bf16, tag="u")
                for p0, p1 in CHUNKS:
                    for i in range(2):
                        nc.sync.dma_start(
                            out=xw[:, i * FB + p0 * PL:i * FB + p1 * PL],
                            in_=xx_d[:, i * FB + p0 * PL:i * FB + p1 * PL])
                for p0, p1 in CHUNKS:
                    n = (p1 - p0) * BB   # rows in chunk
                    for i in range(2):
                        xr = xw[:, i * FB + p0 * PL:i * FB + p1 * PL] \
                            .rearrange("p (r w) -> p r w", w=BB)
                        u21 = U[:, 0:n * 21].rearrange("p (r w) -> p r w",
                                                       w=21)
                        nc.vector.tensor_add(u21[:, :, :],
                                             xr[:, :, 0:21], xr[:, :, 1:22])
                        swd = swf[i][:, p0 * 440:p1 * 440] \
                            .rearrange("p (r w) -> p r w", w=BC)
                        nc.vector.tensor_add(swd[:, :, :],
                                             u21[:, :, 0:20], u21[:, :, 1:21])
                        dwd = dwf[i][:, p0 * 440:p1 * 440] \
                            .rearrange("p (r w) -> p r w", w=BC)
                        nc.vector.tensor_sub(dwd[:, :, :],
                                             xr[:, :, 2:22], xr[:, :, 0:20])

            # ---- slab phase: h/d stages + pointwise, flat dense ops
            with tc.tile_pool(name="work", bufs=1) as work, \
                 tc.tile_pool(name="fld", bufs=1) as fld, \
                 tc.tile_pool(name="late", bufs=1) as late:
                sqa = late.tile([128, 2 * HF], bf16, tag="sq0")
                sqb = late.tile([128, 2 * HF], bf16, tag="sq1")
                sqc = late.tile([128, 2 * HF], bf16, tag="sq2")
                sq = [sqa, sqb, sqc]
                stile = late.tile([128, 2 * HF], bf16, tag="s")
                M12 = late.tile([128, HF], bf16, tag="m12")
                DOT = late.tile([128, HF], bf16, tag="dot")
                Q = late.tile([128, HF], bf16, tag="q")
                RT = late.tile([128, HF], bf16, tag="r")
                SCR = late.tile([128, HF], bf16, tag="scr")
                CT = SCR   # c-mult output reuses scr (disjoint lifetimes)
                LNQ = late.tile([128, HF], f32, tag="lnq")

                def conv_hd(s, mid=None):
                    """h/d stages + squares for one slab (both inputs)."""
                    o = 2200 * s
                    ga = fld.tile([128, 2 * HF], bf16, tag="g0")
                    gb = fld.tile([128, 2 * HF], bf16, tag="g1")
                    gc = fld.tile([128, 2 * HF], bf16, tag="g2")
                    gt = [ga, gb, gc]
                    for i in range(2):
                        if i == 1 and mid is not None:
                            mid()
                        sw, dw = swf[i], dwf[i]
                        W0 = work.tile([128, 3060], bf16, tag="W0")
                        W1 = work.tile([128, 3040], bf16, tag="W1")
                        W2 = work.tile([128, 3060], bf16, tag="W2")
                        W3 = work.tile([128, 3040], bf16, tag="W3")
                        W4 = work.tile([128, 3040], bf16, tag="W4")
                        uh1, dhsw, uh2, shsw, shdw = W0, W1, W2, W3, W4
                        nc.vector.tensor_add(uh1[:, 0:3060],
                                             sw[:, o:o + 3060],
                                             sw[:, o + 20:o + 3080])
                        nc.vector.tensor_sub(dhsw[:, 0:3040],
                                             sw[:, o + 40:o + 3080],
                                             sw[:, o:o + 3040])
                        nc.vector.tensor_add(uh2[:, 0:3060],
                                             dw[:, o:o + 3060],
                                             dw[:, o + 20:o + 3080])
                        nc.vector.tensor_add(shsw[:, 0:3040],
                                             uh1[:, 0:3040], uh1[:, 20:3060])
                        nc.vector.tensor_add(shdw[:, 0:3040],
                                             uh2[:, 0:3040], uh2[:, 20:3060])
                        ud1 = work.tile([128, 2600], bf16, tag="U1")
                        ud2 = work.tile([128, 2600], bf16, tag="U2")
                        nc.vector.tensor_add(ud1[:, 0:2600],
                                             shdw[:, 0:2600],
                                             shdw[:, 440:3040])
                        nc.vector.tensor_add(ud2[:, 0:2600],
                                             dhsw[:, 0:2600],
                                             dhsw[:, 440:3040])
                        gx = gt[0][:, i * HF:(i + 1) * HF]
                        gy = gt[1][:, i * HF:(i + 1) * HF]
                        gz = gt[2][:, i * HF:(i + 1) * HF]
                        nc.vector.tensor_add(gx[:, 0:2160],
                                             ud1[:, 0:2160], ud1[:, 440:2600])
                        nc.vector.tensor_add(gy[:, 0:2160],
                                             ud2[:, 0:2160], ud2[:, 440:2600])
                        nc.vector.tensor_sub(gz[:, 0:2160],
                                             shsw[:, 880:3040],
                                             shsw[:, 0:2160])
                        # junk-row memsets on GpSimd: frees ~5us of
                        # VectorE queue; too brief to hit the DVE port
                        # conflict.
                        for g, val in ((gx, 0.0), (gy, 0.0), (gz, 1.0)):
                            jv = g.rearrange("p (r w) -> p r w", w=440)
                            nc.gpsimd.memset(jv[:, :, 400:440], val)
                        for k, col in ((0, 0), (1, 1), (2, 2)):
                            gh = gt[k][:, i * HF:(i + 1) * HF]
                            sh = sq[k][:, i * HF:(i + 1) * HF]
                            nc.scalar.activation(
                                sh[:, 0:HF], gh[:, 0:HF], AF.Square,
                                accum_out=acc[:, 8 * s + 2 * col + i:
                                              8 * s + 2 * col + i + 1])
                    return gt

                def pw_a_v(F, e0=0, e1=HF):
                    """V: dot products, s sums, q on flat [e0:e1)."""
                    MT = (Q, RT, SCR)
                    for m, g in zip(MT, F):
                        nc.vector.tensor_mul(m[:, e0:e1],
                                             g[:, e0:e1],
                                             g[:, HF + e0:HF + e1])
                    nc.vector.tensor_add(M12[:, e0:e1],
                                         MT[0][:, e0:e1], MT[1][:, e0:e1])
                    nc.vector.tensor_add(DOT[:, e0:e1],
                                         M12[:, e0:e1], MT[2][:, e0:e1])
                    for i in range(2):
                        h0 = i * HF
                        nc.vector.tensor_add(M12[:, e0:e1],
                                             sq[0][:, h0 + e0:h0 + e1],
                                             sq[1][:, h0 + e0:h0 + e1])
                        nc.vector.tensor_add(stile[:, h0 + e0:h0 + e1],
                                             M12[:, e0:e1],
                                             sq[2][:, h0 + e0:h0 + e1])
                    nc.vector.tensor_mul(Q[:, e0:e1],
                                         stile[:, e0:e1],
                                         stile[:, HF + e0:HF + e1])

                def pw_a_ln(e0=0, e1=HF):
                    nc.scalar.activation(LNQ[:, e0:e1], Q[:, e0:e1], AF.Ln)

                def pw_a_exp(cb, e0=0, e1=HF):
                    # r first: the downstream c-mult waits only on r
                    nc.scalar.activation(RT[:, e0:e1], LNQ[:, e0:e1],
                                         AF.Exp, scale=-0.5)
                    nc.scalar.activation(SCR[:, e0:e1], LNQ[:, e0:e1],
                                         AF.Exp, scale=0.5,
                                         accum_out=acc[:, 32 + cb:33 + cb])

                def pw_b_v(e0=0, e1=HF):
                    nc.vector.tensor_mul(CT[:, e0:e1],
                                         DOT[:, e0:e1], RT[:, e0:e1])

                def pw_b_s(cb, e0=0, e1=HF):
                    nc.scalar.activation(M12[:, e0:e1], CT[:, e0:e1],
                                         AF.Copy,
                                         accum_out=acc[:, 40 + cb:41 + cb])

                def pw_b_vred(cb, e0, e1):
                    nc.vector.tensor_mul(CT[:, e0:e1],
                                         DOT[:, e0:e1], RT[:, e0:e1])
                    nc.vector.tensor_reduce(acc[:, 40 + cb:41 + cb],
                                            CT[:, e0:e1],
                                            axis=mybir.AxisListType.X,
                                            op=mybir.AluOpType.add)

                F0 = conv_hd(0)
                pw_a_v(F0)
                pw_a_ln()
                pw_a_exp(0)
                F1 = conv_hd(1, mid=lambda: (pw_b_v(), pw_b_s(0)))
                pw_a_v(F1)
                pw_a_ln()
                pw_a_exp(1)
                F2 = conv_hd(2, mid=lambda: (pw_b_v(), pw_b_s(1)))
                pw_a_v(F2)
                pw_a_ln()
                pw_a_exp(2)
                F3 = conv_hd(3, mid=lambda: (pw_b_v(), pw_b_s(2)))
                HBE = 1100
                pw_a_v(F3, 0, HBE)
                pw_a_v(F3, HBE, HF)
                pw_a_ln(0, HBE)
                pw_a_ln(HBE, HF)
                pw_a_exp(3, 0, HBE)
                pw_b_vred(3, 0, HBE)
                pw_a_exp(4, HBE, HF)
                pw_b_vred(4, HBE, HF)

                nc.sync.dma_start(out=out_d[:], in_=acc[:])

    nc.compile()
    return nc


def _shard_inputs(pred, target):
    bf = ml_dtypes.bfloat16
    starts = np.arange(0, 160, BC)

    blocked = {}
    for name, x in (("pred", pred), ("targ", target)):
        per_b = []
        for b in range(2):
            gp = np.zeros((162, 162, 162), np.float32)
            gp[1:161, 1:161, 1:161] = x[b, 0]
            swv = np.lib.stride_tricks.sliding_window_view(gp, (BB, BB, BB))
            per_b.append(swv)
        blocked[name] = per_b

    in_maps = []
    for core in range(N_CORES):
        b, q = divmod(core, 4)
        xx = np.empty((128, 2 * FB), bf)
        for i, name in enumerate(("pred", "targ")):
            swv = blocked[name][b]
            blk = swv[np.ix_([40 * q, 40 * q + BC], starts, starts)]
            xx[:, i * FB:(i + 1) * FB] = blk.reshape(128, FB).astype(bf)
        in_maps.append({"xx": xx})
    return in_maps


def run(pred, target, trace=False):
    from concourse.bass_utils import run_bass_kernel_spmd

    pred = np.asarray(pred, dtype=np.float32)
    target = np.asarray(target, dtype=np.float32)
    assert pred.shape == (2, 1, 160, 160, 160)

    if "nc" not in _cache:
        _cache["nc"] = _build()
    nc = _cache["nc"]

    in_maps = _shard_inputs(pred, target)
    res = None
    for attempt in range(3):
        try:
            res = run_bass_kernel_spmd(
                nc, in_maps, core_ids=list(range(N_CORES)), trace=trace)
            break
        except Exception:
            if attempt == 2:
                raise
            import time as _time
            _time.sleep(5)

    sq_sum = 0.0
    sqrt_sum = 0.0
    c_sum = 0.0
    for core_out in res.results:
        o = np.asarray(core_out["out"], np.float64)
        for s in range(NSLAB):
            sq_sum += o[:, 8 * s:8 * s + 6].sum()
        sqrt_sum += o[:, 32:37].sum()
        c_sum += o[:, 40:45].sum()

    junk = float(JUNK_PER_SLAB * 128 * NSLAB * N_CORES)
    mag_sum = (sq_sum - 2 * junk) - 2.0 * (sqrt_sum - junk)
    loss = WEIGHT * (mag_sum / NVOX + 1.0 - (c_sum - junk) / NVOX)
    return np.float32(loss), res.exec_time_ns


def kernel(pred, target):
    loss, _ = run(pred, target, trace=False)
    return loss


# revision 4
# speedup vs baseline: 1.1791x; 1.1791x over previous
"""Trainium2 distributed kernel for AnatomicalConsistencyLoss, v3.

Like v2 (see kernel_v2.py docstring for the engine-assignment rationale:
VectorE does all tensor_tensor work in the 2x bf16 mode, ScalarE all
unaries + accum reductions, other engines measured counterproductive),
plus:

- The w-stage (S_w / D_w) runs ONCE over the full 22-plane input volume
  instead of per-slab (saves the 2-plane slab halo reprocessing), tiled
  by DMA chunk for overlap with the input transfer.
- sw/dw and everything downstream are stored with dense 20-wide rows,
  so all h/d-stage and pointwise ops are flat 1-D contiguous slices
  (junk h-rows remain, zeroed/one'd in the field tiles and exactly
  subtracted host-side).
- The input tile and u-scratch live in a pool closed after the w-stage
  so the slab-phase tiles reuse their SBUF.
"""

import sys

import numpy as np

sys.path.insert(0, "/opt/trn_rl_repo")

import ml_dtypes

N_CORES = 8
BC = 20
BB = 22
PL = BB * BB     # 484
FB = BB ** 3     # 10648
NVOX = 2 * 160 * 160 * 160
WEIGHT = 0.2

SD = 5
NSLAB = 4
HF = SD * BC * BB    # 2200: dense field half size
NR = SD * BB         # 110 rows per half
JUNK_PER_SLAB = 10 * BC

# DMA/w-stage chunks in planes
CHUNKS = [(0, 3), (3, 8), (8, 13), (13, 17), (17, 22)]

_cache = {}


def _build():
    import concourse.bacc as bacc
    import concourse.tile as tile
    from concourse import mybir

    f32 = mybir.dt.float32
    bf16 = mybir.dt.bfloat16
    AF = mybir.ActivationFunctionType

    nc = bacc.Bacc(
        "TRN2",
        target_bir_lowering=False,
        debug=False,
        enable_asserts=False,
        num_devices=N_CORES,
    )
    xx_d = nc.dram_tensor("xx", [128, 2 * FB], bf16, kind="ExternalInput")
    out_d = nc.dram_tensor("out", [128, 48], f32, kind="ExternalOutput")

    with tile.TileContext(nc) as tc:
        with tc.tile_pool(name="pers", bufs=1) as pers:
            acc = pers.tile([128, 48], f32, tag="acc")
            sw0 = pers.tile([128, 9680], bf16, tag="sw0")
            sw1 = pers.tile([128, 9680], bf16, tag="sw1")
            dw0 = pers.tile([128, 9680], bf16, tag="dw0")
            dw1 = pers.tile([128, 9680], bf16, tag="dw1")
            swf = [sw0, sw1]
            dwf = [dw0, dw1]

            # ---- w-stage over the full volume, chunked by DMA arrival
            with tc.tile_pool(name="xp", bufs=1) as xp:
                xw = xp.tile([128, 2 * FB], bf16, tag="xw")
                U = xp.tile([128, 132 * 21], bf16, tag="u")
                for p0, p1 in CHUNKS:
                    for i in range(2):
                        nc.sync.dma_start(
                            out=xw[:, i * FB + p0 * PL:i * FB + p1 * PL],
                            in_=xx_d[:, i * FB + p0 * PL:i * FB + p1 * PL])
                for p0, p1 in CHUNKS:
                    n = (p1 - p0) * BB   # rows in chunk
                    for i in range(2):
                        xr = xw[:, i * FB + p0 * PL:i * FB + p1 * PL] \
                            .rearrange("p (r w) -> p r w", w=BB)
                        u21 = U[:, 0:n * 21].rearrange("p (r w) -> p r w",
                                                       w=21)
                        nc.vector.tensor_add(u21[:, :, :],
                                             xr[:, :, 0:21], xr[:, :, 1:22])
                        swd = swf[i][:, p0 * 440:p1 * 440] \
                            .rearrange("p (r w) -> p r w", w=BC)
                        nc.vector.tensor_add(swd[:, :, :],
                                             u21[:, :, 0:20], u21[:, :, 1:21])
                        dwd = dwf[i][:, p0 * 440:p1 * 440] \
                            .rearrange("p (r w) -> p r w", w=BC)
                        nc.vector.tensor_sub(dwd[:, :, :],
                                             xr[:, :, 2:22], xr[:, :, 0:20])

            # ---- slab phase: h/d stages + pointwise, flat dense ops
            with tc.tile_pool(name="work", bufs=1) as work, \
                 tc.tile_pool(name="fld", bufs=1) as fld, \
                 tc.tile_pool(name="late", bufs=1) as late:
                sqa = late.tile([128, 2 * HF], bf16, tag="sq0")
                sqb = late.tile([128, 2 * HF], bf16, tag="sq1")
                sqc = late.tile([128, 2 * HF], bf16, tag="sq2")
                sq = [sqa, sqb, sqc]
                stile = late.tile([128, 2 * HF], bf16, tag="s")
                M12 = late.tile([128, HF], bf16, tag="m12")
                DOT = late.tile([128, HF], bf16, tag="dot")
                Q = late.tile([128, HF], bf16, tag="q")
                RT = late.tile([128, HF], bf16, tag="r")
                SCR = late.tile([128, HF], bf16, tag="scr")
                CT = SCR   # c-mult output reuses scr (disjoint lifetimes)
                LNQ = late.tile([128, HF], f32, tag="lnq")

                def conv_hd(s, mid=None):
                    """h/d stages + squares for one slab (both inputs)."""
                    o = 2200 * s
                    ga = fld.tile([128, 2 * HF], bf16, tag="g0")
                    gb = fld.tile([128, 2 * HF], bf16, tag="g1")
                    gc = fld.tile([128, 2 * HF], bf16, tag="g2")
                    gt = [ga, gb, gc]
                    for i in range(2):
                        if i == 1 and mid is not None:
                            mid()
                        sw, dw = swf[i], dwf[i]
                        W0 = work.tile([128, 3060], bf16, tag="W0")
                        W1 = work.tile([128, 3040], bf16, tag="W1")
                        W2 = work.tile([128, 3060], bf16, tag="W2")
                        W3 = work.tile([128, 3040], bf16, tag="W3")
                        W4 = work.tile([128, 3040], bf16, tag="W4")
                        uh1, dhsw, uh2, shsw, shdw = W0, W1, W2, W3, W4
                        nc.vector.tensor_add(uh1[:, 0:3060],
                                             sw[:, o:o + 3060],
                                             sw[:, o + 20:o + 3080])
                        nc.vector.tensor_sub(dhsw[:, 0:3040],
                                             sw[:, o + 40:o + 3080],
                                             sw[:, o:o + 3040])
                        nc.vector.tensor_add(uh2[:, 0:3060],
                                             dw[:, o:o + 3060],
                                             dw[:, o + 20:o + 3080])
                        nc.vector.tensor_add(shsw[:, 0:3040],
                                             uh1[:, 0:3040], uh1[:, 20:3060])
                        nc.vector.tensor_add(shdw[:, 0:3040],
                                             uh2[:, 0:3040], uh2[:, 20:3060])
                        ud1 = work.tile([128, 2600], bf16, tag="U1")
                        ud2 = work.tile([128, 2600], bf16, tag="U2")
                        nc.vector.tensor_add(ud1[:, 0:2600],
                                             shdw[:, 0:2600],
                                             shdw[:, 440:3040])
                        nc.vector.tensor_add(ud2[:, 0:2600],
                                             dhsw[:, 0:2600],
                                             dhsw[:, 440:3040])
                        gx = gt[0][:, i * HF:(i + 1) * HF]
                        gy = gt[1][:, i * HF:(i + 1) * HF]
                        gz = gt[2][:, i * HF:(i + 1) * HF]
                        nc.vector.tensor_add(gx[:, 0:2160],
                                             ud1[:, 0:2160], ud1[:, 440:2600])
                        nc.vector.tensor_add(gy[:, 0:2160],
                                             ud2[:, 0:2160], ud2[:, 440:2600])
                        nc.vector.tensor_sub(gz[:, 0:2160],
                                             shsw[:, 880:3040],
                                             shsw[:, 0:2160])
                        for g, val in ((gx, 0.0), (gy, 0.0), (gz, 1.0)):
                            jv = g.rearrange("p (r w) -> p r w", w=440)
                            nc.vector.memset(jv[:, :, 400:440], val)
                        for k, col in ((0, 0), (1, 1), (2, 2)):
                            gh = gt[k][:, i * HF:(i + 1) * HF]
                            sh = sq[k][:, i * HF:(i + 1) * HF]
                            nc.scalar.activation(
                                sh[:, 0:HF], gh[:, 0:HF], AF.Square,
                                accum_out=acc[:, 8 * s + 2 * col + i:
                                              8 * s + 2 * col + i + 1])
                    return gt

                def pw_a_v(F, e0=0, e1=HF):
                    """V: dot products, s sums, q on flat [e0:e1)."""
                    MT = (Q, RT, SCR)
                    for m, g in zip(MT, F):
                        nc.vector.tensor_mul(m[:, e0:e1],
                                             g[:, e0:e1],
                                             g[:, HF + e0:HF + e1])
                    nc.vector.tensor_add(M12[:, e0:e1],
                                         MT[0][:, e0:e1], MT[1][:, e0:e1])
                    nc.vector.tensor_add(DOT[:, e0:e1],
                                         M12[:, e0:e1], MT[2][:, e0:e1])
                    for i in range(2):
                        h0 = i * HF
                        nc.vector.tensor_add(M12[:, e0:e1],
                                             sq[0][:, h0 + e0:h0 + e1],
                                             sq[1][:, h0 + e0:h0 + e1])
                        nc.vector.tensor_add(stile[:, h0 + e0:h0 + e1],
                                             M12[:, e0:e1],
                                             sq[2][:, h0 + e0:h0 + e1])
                    nc.vector.tensor_mul(Q[:, e0:e1],
                                         stile[:, e0:e1],
                                         stile[:, HF + e0:HF + e1])

                def pw_a_ln(e0=0, e1=HF):
                    nc.scalar.activation(LNQ[:, e0:e1], Q[:, e0:e1], AF.Ln)

                def pw_a_exp(cb, e0=0, e1=HF):
                    # r first: the downstream c-mult waits only on r
                    nc.scalar.activation(RT[:, e0:e1], LNQ[:, e0:e1],
                                         AF.Exp, scale=-0.5)
                    nc.scalar.activation(SCR[:, e0:e1], LNQ[:, e0:e1],
                                         AF.Exp, scale=0.5,
                                         accum_out=acc[:, 32 + cb:33 + cb])

                def pw_b_v(e0=0, e1=HF):
                    nc.vector.tensor_mul(CT[:, e0:e1],
                                         DOT[:, e0:e1], RT[:, e0:e1])

                def pw_b_s(cb, e0=0, e1=HF):
                    nc.scalar.activation(M12[:, e0:e1], CT[:, e0:e1],
                                         AF.Copy,
                                         accum_out=acc[:, 40 + cb:41 + cb])

                def pw_b_vred(cb, e0, e1):
                    nc.vector.tensor_mul(CT[:, e0:e1],
                                         DOT[:, e0:e1], RT[:, e0:e1])
                    nc.vector.tensor_reduce(acc[:, 40 + cb:41 + cb],
                                            CT[:, e0:e1],
                                            axis=mybir.AxisListType.X,
                                            op=mybir.AluOpType.add)

                F0 = conv_hd(0)
                pw_a_v(F0)
                pw_a_ln()
                pw_a_exp(0)
                F1 = conv_hd(1, mid=lambda: (pw_b_v(), pw_b_s(0)))
                pw_a_v(F1)
                pw_a_ln()
                pw_a_exp(1)
                F2 = conv_hd(2, mid=lambda: (pw_b_v(), pw_b_s(1)))
                pw_a_v(F2)
                pw_a_ln()
                pw_a_exp(2)
                F3 = conv_hd(3, mid=lambda: (pw_b_v(), pw_b_s(2)))
                HBE = 1100
                pw_a_v(F3, 0, HBE)
                pw_a_v(F3, HBE, HF)
                pw_a_ln(0, HBE)
                pw_a_ln(HBE, HF)
                pw_a_exp(3, 0, HBE)
                pw_b_vred(3, 0, HBE)
                pw_a_exp(4, HBE, HF)
                pw_b_vred(4, HBE, HF)

                nc.sync.dma_start(out=out_d[:], in_=acc[:])

    nc.compile()
    return nc


def _shard_inputs(pred, target):
    bf = ml_dtypes.bfloat16
    starts = np.arange(0, 160, BC)

    blocked = {}
    for name, x in (("pred", pred), ("targ", target)):
        per_b = []
        for b in range(2):
            gp = np.zeros((162, 162, 162), np.float32)
            gp[1:161, 1:161, 1:161] = x[b, 0]
            swv = np.lib.stride_tricks.sliding_window_view(gp, (BB, BB, BB))
            per_b.append(swv)
        blocked[name] = per_b

    in_maps = []
    for core in range(N_CORES):
        b, q = divmod(core, 4)
        xx = np.empty((128, 2 * FB), bf)
        for i, name in enumerate(("pred", "targ")):
            swv = blocked[name][b]
            blk = swv[np.ix_([40 * q, 40 * q + BC], starts, starts)]
            xx[:, i * FB:(i + 1) * FB] = blk.reshape(128, FB).astype(bf)
        in_maps.append({"xx": xx})
    return in_maps


def run(pred, target, trace=False):
    from concourse.bass_utils import run_bass_kernel_spmd

    pred = np.asarray(pred, dtype=np.float32)
    target = np.asarray(target, dtype=np.float32)
    assert pred.shape == (2, 1, 160, 160, 160)

    if "nc" not in _cache:
        _cache["nc"] = _build()
    nc = _cache["nc"]

    in_maps = _shard_inputs(pred, target)
    res = None
    for attempt in range(3):
        try:
            res = run_bass_kernel_spmd(
                nc, in_maps, core_ids=list(range(N_CORES)), trace=trace)
            break
        except Exception:
            if attempt == 2:
                raise
            import time as _time
            _time.sleep(5)

    sq_sum = 0.0
    sqrt_sum = 0.0
    c_sum = 0.0
    for core_out in res.results:
        o = np.asarray(core_out["out"], np.float64)
        for s in range(NSLAB):
            sq_sum += o[:, 8 * s:8 * s + 6].sum()
        sqrt_sum += o[:, 32:37].sum()
        c_sum += o[:, 40:45].sum()

    junk = float(JUNK_PER_SLAB * 128 * NSLAB * N_CORES)
    mag_sum = (sq_sum - 2 * junk) - 2.0 * (sqrt_sum - junk)
    loss = WEIGHT * (mag_sum / NVOX + 1.0 - (c_sum - junk) / NVOX)
    return np.float32(loss), res.exec_time_ns


def kernel(pred, target):
    loss, _ = run(pred, target, trace=False)
    return loss


# revision 5
# speedup vs baseline: 1.2502x; 1.0603x over previous
"""Trainium2 distributed kernel for AnatomicalConsistencyLoss, v3.

Like v2 (see kernel_v2.py docstring for the engine-assignment rationale:
VectorE does all tensor_tensor work in the 2x bf16 mode, ScalarE all
unaries + accum reductions, other engines measured counterproductive),
plus:

- The w-stage (S_w / D_w) runs ONCE over the full 22-plane input volume
  instead of per-slab (saves the 2-plane slab halo reprocessing), tiled
  by DMA chunk for overlap with the input transfer.
- sw/dw and everything downstream are stored with dense 20-wide rows,
  so all h/d-stage and pointwise ops are flat 1-D contiguous slices
  (junk h-rows remain, zeroed/one'd in the field tiles and exactly
  subtracted host-side).
- The input tile and u-scratch live in a pool closed after the w-stage
  so the slab-phase tiles reuse their SBUF.
"""

import sys

import numpy as np

sys.path.insert(0, "/opt/trn_rl_repo")

import ml_dtypes

N_CORES = 8
BC = 20
BB = 22
PL = BB * BB     # 484
FB = BB ** 3     # 10648
NVOX = 2 * 160 * 160 * 160
WEIGHT = 0.2

SD = 5
NSLAB = 4
HF = SD * BC * BB    # 2200: dense field half size
NR = SD * BB         # 110 rows per half
JUNK_PER_SLAB = 10 * BC

# DMA/w-stage chunks in planes
CHUNKS = [(0, 3), (3, 8), (8, 13), (13, 17), (17, 22)]

_cache = {}


def _build():
    import concourse.bacc as bacc
    import concourse.tile as tile
    from concourse import mybir

    f32 = mybir.dt.float32
    bf16 = mybir.dt.bfloat16
    AF = mybir.ActivationFunctionType

    nc = bacc.Bacc(
        "TRN2",
        target_bir_lowering=False,
        debug=False,
        enable_asserts=False,
        num_devices=N_CORES,
    )
    xx_d = nc.dram_tensor("xx", [128, 2 * FB], bf16, kind="ExternalInput")
    out_d = nc.dram_tensor("out", [128, 48], f32, kind="ExternalOutput")

    with tile.TileContext(nc) as tc:
        with tc.tile_pool(name="pers", bufs=1) as pers:
            acc = pers.tile([128, 48], f32, tag="acc")
            sw0 = pers.tile([128, 9680], bf16, tag="sw0")
            sw1 = pers.tile([128, 9680], bf16, tag="sw1")
            dw0 = pers.tile([128, 9680], bf16, tag="dw0")
            dw1 = pers.tile([128, 9680], bf16, tag="dw1")
            swf = [sw0, sw1]
            dwf = [dw0, dw1]

            # ---- w-stage over the full volume, chunked by DMA arrival
            with tc.tile_pool(name="xp", bufs=1) as xp:
                xw = xp.tile([128, 2 * FB], bf16, tag="xw")
                U = xp.tile([128, 132 * 21], bf16, tag="u")
                for p0, p1 in CHUNKS:
                    for i in range(2):
                        nc.sync.dma_start(
                            out=xw[:, i * FB + p0 * PL:i * FB + p1 * PL],
                            in_=xx_d[:, i * FB + p0 * PL:i * FB + p1 * PL])
                for p0, p1 in CHUNKS:
                    n = (p1 - p0) * BB   # rows in chunk
                    for i in range(2):
                        xr = xw[:, i * FB + p0 * PL:i * FB + p1 * PL] \
                            .rearrange("p (r w) -> p r w", w=BB)
                        u21 = U[:, 0:n * 21].rearrange("p (r w) -> p r w",
                                                       w=21)
                        nc.vector.tensor_add(u21[:, :, :],
                                             xr[:, :, 0:21], xr[:, :, 1:22])
                        swd = swf[i][:, p0 * 440:p1 * 440] \
                            .rearrange("p (r w) -> p r w", w=BC)
                        nc.vector.tensor_add(swd[:, :, :],
                                             u21[:, :, 0:20], u21[:, :, 1:21])
                        dwd = dwf[i][:, p0 * 440:p1 * 440] \
                            .rearrange("p (r w) -> p r w", w=BC)
                        nc.vector.tensor_sub(dwd[:, :, :],
                                             xr[:, :, 2:22], xr[:, :, 0:20])

            # ---- slab phase: h/d stages + pointwise, flat dense ops
            with tc.tile_pool(name="work", bufs=1) as work, \
                 tc.tile_pool(name="fld", bufs=1) as fld, \
                 tc.tile_pool(name="late", bufs=1) as late:
                sqa = late.tile([128, 2 * HF], bf16, tag="sq0")
                sqb = late.tile([128, 2 * HF], bf16, tag="sq1")
                sqc = late.tile([128, 2 * HF], bf16, tag="sq2")
                sq = [sqa, sqb, sqc]
                stile = late.tile([128, 2 * HF], bf16, tag="s")
                M12 = late.tile([128, HF], bf16, tag="m12")
                DOT = late.tile([128, HF], bf16, tag="dot")
                Q = late.tile([128, HF], bf16, tag="q")
                RT = late.tile([128, HF], bf16, tag="r")
                SCR = late.tile([128, HF], bf16, tag="scr")
                CT = SCR   # c-mult output reuses scr (disjoint lifetimes)
                LNQ = late.tile([128, HF], f32, tag="lnq")

                def pv(t, off, np_, a, b, w=440):
                    """Per-plane valid view: [[w, np_],[1, b-a]]."""
                    return t[:, off:off + np_ * w] \
                        .rearrange("p (r w) -> p r w", w=w)[:, :, a:b]

                def conv_hd(s, mid=None):
                    """h/d stages + squares for one slab (both inputs).
                    Per-plane views skip the cross-plane junk h-rows
                    entirely: no junk computed, read, or corrected."""
                    o = 2200 * s
                    ga = fld.tile([128, 2 * HF], bf16, tag="g0")
                    gb = fld.tile([128, 2 * HF], bf16, tag="g1")
                    gc = fld.tile([128, 2 * HF], bf16, tag="g2")
                    gt = [ga, gb, gc]
                    for i in range(2):
                        if i == 1 and mid is not None:
                            mid()
                        sw, dw = swf[i], dwf[i]
                        W0 = work.tile([128, 3060], bf16, tag="W0")
                        W1 = work.tile([128, 3040], bf16, tag="W1")
                        W2 = work.tile([128, 3060], bf16, tag="W2")
                        W3 = work.tile([128, 3040], bf16, tag="W3")
                        W4 = work.tile([128, 3040], bf16, tag="W4")
                        uh1, dhsw, uh2, shsw, shdw = W0, W1, W2, W3, W4
                        nc.vector.tensor_add(pv(uh1, 0, 7, 0, 420, 420),
                                             pv(sw, o, 7, 0, 420),
                                             pv(sw, o, 7, 20, 440))
                        nc.vector.tensor_sub(pv(dhsw, 0, 7, 0, 400, 400),
                                             pv(sw, o, 7, 40, 440),
                                             pv(sw, o, 7, 0, 400))
                        nc.vector.tensor_add(pv(uh2, 0, 7, 0, 420, 420),
                                             pv(dw, o, 7, 0, 420),
                                             pv(dw, o, 7, 20, 440))
                        nc.vector.tensor_add(pv(shsw, 0, 7, 0, 400, 400),
                                             pv(uh1, 0, 7, 0, 400, 420),
                                             pv(uh1, 0, 7, 20, 420, 420))
                        nc.vector.tensor_add(pv(shdw, 0, 7, 0, 400, 400),
                                             pv(uh2, 0, 7, 0, 400, 420),
                                             pv(uh2, 0, 7, 20, 420, 420))
                        ud1 = work.tile([128, 2600], bf16, tag="U1")
                        ud2 = work.tile([128, 2600], bf16, tag="U2")
                        nc.vector.tensor_add(ud1[:, 0:2400],
                                             shdw[:, 0:2400],
                                             shdw[:, 400:2800])
                        nc.vector.tensor_add(ud2[:, 0:2400],
                                             dhsw[:, 0:2400],
                                             dhsw[:, 400:2800])
                        h0 = i * HF
                        nc.vector.tensor_add(gt[0][:, h0:h0 + 2000],
                                             ud1[:, 0:2000], ud1[:, 400:2400])
                        nc.vector.tensor_add(gt[1][:, h0:h0 + 2000],
                                             ud2[:, 0:2000], ud2[:, 400:2400])
                        nc.vector.tensor_sub(gt[2][:, h0:h0 + 2000],
                                             shsw[:, 800:2800], shsw[:, 0:2000])
                        for k in range(3):
                            nc.scalar.activation(
                                sq[k][:, h0:h0 + 2000],
                                gt[k][:, h0:h0 + 2000], AF.Square,
                                accum_out=acc[:, 8 * s + 2 * k + i:
                                              8 * s + 2 * k + i + 1])
                    return gt

                def pw_a_v(F, p0=0, np_=5):
                    """V: dot products, s sums, q on planes [p0, p0+np_)."""
                    a, b = p0 * 400, (p0 + np_) * 400
                    MT = (Q, RT, SCR)
                    for m, g in zip(MT, F):
                        nc.vector.tensor_mul(m[:, a:b], g[:, a:b],
                                             g[:, HF + a:HF + b])
                    nc.vector.tensor_add(M12[:, a:b],
                                         MT[0][:, a:b], MT[1][:, a:b])
                    nc.vector.tensor_add(DOT[:, a:b],
                                         M12[:, a:b], MT[2][:, a:b])
                    for i in range(2):
                        h0 = i * HF
                        nc.vector.tensor_add(M12[:, a:b],
                                             sq[0][:, h0 + a:h0 + b],
                                             sq[1][:, h0 + a:h0 + b])
                        nc.vector.tensor_add(stile[:, h0 + a:h0 + b],
                                             M12[:, a:b],
                                             sq[2][:, h0 + a:h0 + b])
                    nc.vector.tensor_mul(Q[:, a:b], stile[:, a:b],
                                         stile[:, HF + a:HF + b])

                def pw_a_ln(p0=0, np_=5):
                    a, b = p0 * 400, (p0 + np_) * 400
                    nc.scalar.activation(LNQ[:, a:b], Q[:, a:b], AF.Ln)

                def pw_a_exp(cb, p0=0, np_=5):
                    a, b = p0 * 400, (p0 + np_) * 400
                    # r first: the downstream c-mult waits only on r
                    nc.scalar.activation(RT[:, a:b], LNQ[:, a:b],
                                         AF.Exp, scale=-0.5)
                    nc.scalar.activation(SCR[:, a:b], LNQ[:, a:b],
                                         AF.Exp, scale=0.5,
                                         accum_out=acc[:, 32 + cb:33 + cb])

                def pw_b_v(p0=0, np_=5):
                    a, b = p0 * 400, (p0 + np_) * 400
                    nc.vector.tensor_mul(CT[:, a:b], DOT[:, a:b], RT[:, a:b])

                def pw_b_s(cb, p0=0, np_=5):
                    a, b = p0 * 400, (p0 + np_) * 400
                    nc.scalar.activation(M12[:, a:b], CT[:, a:b], AF.Copy,
                                         accum_out=acc[:, 40 + cb:41 + cb])

                def pw_b_vred(cb, p0, np_):
                    a, b = p0 * 400, (p0 + np_) * 400
                    nc.vector.tensor_mul(CT[:, a:b], DOT[:, a:b], RT[:, a:b])
                    nc.vector.tensor_reduce(acc[:, 40 + cb:41 + cb],
                                            CT[:, a:b],
                                            axis=mybir.AxisListType.X,
                                            op=mybir.AluOpType.add)

                F0 = conv_hd(0)
                pw_a_v(F0)
                pw_a_ln()
                pw_a_exp(0)
                F1 = conv_hd(1, mid=lambda: (pw_b_v(), pw_b_s(0)))
                pw_a_v(F1)
                pw_a_ln()
                pw_a_exp(1)
                F2 = conv_hd(2, mid=lambda: (pw_b_v(), pw_b_s(1)))
                pw_a_v(F2)
                pw_a_ln()
                pw_a_exp(2)
                F3 = conv_hd(3, mid=lambda: (pw_b_v(), pw_b_s(2)))
                pw_a_v(F3, 0, 3)
                pw_a_v(F3, 3, 2)
                pw_a_ln(0, 3)
                pw_a_ln(3, 2)
                pw_a_exp(3, 0, 3)
                pw_b_vred(3, 0, 3)
                pw_a_exp(4, 3, 2)
                pw_b_vred(4, 3, 2)

                nc.sync.dma_start(out=out_d[:], in_=acc[:])

    nc.compile()
    return nc


def _shard_inputs(pred, target):
    bf = ml_dtypes.bfloat16
    starts = np.arange(0, 160, BC)

    blocked = {}
    for name, x in (("pred", pred), ("targ", target)):
        per_b = []
        for b in range(2):
            gp = np.zeros((162, 162, 162), np.float32)
            gp[1:161, 1:161, 1:161] = x[b, 0]
            swv = np.lib.stride_tricks.sliding_window_view(gp, (BB, BB, BB))
            per_b.append(swv)
        blocked[name] = per_b

    in_maps = []
    for core in range(N_CORES):
        b, q = divmod(core, 4)
        xx = np.empty((128, 2 * FB), bf)
        for i, name in enumerate(("pred", "targ")):
            swv = blocked[name][b]
            blk = swv[np.ix_([40 * q, 40 * q + BC], starts, starts)]
            xx[:, i * FB:(i + 1) * FB] = blk.reshape(128, FB).astype(bf)
        in_maps.append({"xx": xx})
    return in_maps


def run(pred, target, trace=False):
    from concourse.bass_utils import run_bass_kernel_spmd

    pred = np.asarray(pred, dtype=np.float32)
    target = np.asarray(target, dtype=np.float32)
    assert pred.shape == (2, 1, 160, 160, 160)

    if "nc" not in _cache:
        _cache["nc"] = _build()
    nc = _cache["nc"]

    in_maps = _shard_inputs(pred, target)
    res = None
    for attempt in range(3):
        try:
            res = run_bass_kernel_spmd(
                nc, in_maps, core_ids=list(range(N_CORES)), trace=trace)
            break
        except Exception:
            if attempt == 2:
                raise
            import time as _time
            _time.sleep(5)

    sq_sum = 0.0
    sqrt_sum = 0.0
    c_sum = 0.0
    for core_out in res.results:
        o = np.asarray(core_out["out"], np.float64)
        for s in range(NSLAB):
            sq_sum += o[:, 8 * s:8 * s + 6].sum()
        sqrt_sum += o[:, 32:37].sum()
        c_sum += o[:, 40:45].sum()

    mag_sum = sq_sum - 2.0 * sqrt_sum
    loss = WEIGHT * (mag_sum / NVOX + 1.0 - c_sum / NVOX)
    return np.float32(loss), res.exec_time_ns


def kernel(pred, target):
    loss, _ = run(pred, target, trace=False)
    return loss


# revision 6
# speedup vs baseline: 1.2913x; 1.0329x over previous
"""Trainium2 distributed kernel for AnatomicalConsistencyLoss, v3.

Like v2 (see kernel_v2.py docstring for the engine-assignment rationale:
VectorE does all tensor_tensor work in the 2x bf16 mode, ScalarE all
unaries + accum reductions, other engines measured counterproductive),
plus:

- The w-stage (S_w / D_w) runs ONCE over the full 22-plane input volume
  instead of per-slab (saves the 2-plane slab halo reprocessing), tiled
  by DMA chunk for overlap with the input transfer.
- sw/dw and everything downstream are stored with dense 20-wide rows,
  so all h/d-stage and pointwise ops are flat 1-D contiguous slices
  (junk h-rows remain, zeroed/one'd in the field tiles and exactly
  subtracted host-side).
- The input tile and u-scratch live in a pool closed after the w-stage
  so the slab-phase tiles reuse their SBUF.
"""

import sys

import numpy as np

sys.path.insert(0, "/opt/trn_rl_repo")

import ml_dtypes

N_CORES = 8
BC = 20
BB = 22
PL = BB * BB     # 484
FB = BB ** 3     # 10648
NVOX = 2 * 160 * 160 * 160
WEIGHT = 0.2

SD = 5
NSLAB = 4
HF = SD * BC * BB    # 2200: dense field half size
NR = SD * BB         # 110 rows per half
JUNK_PER_SLAB = 10 * BC

# DMA/w-stage chunks in planes
CHUNKS = [(0, 3), (3, 8), (8, 13), (13, 17), (17, 22)]

_cache = {}


def _build():
    import concourse.bacc as bacc
    import concourse.tile as tile
    from concourse import mybir

    f32 = mybir.dt.float32
    bf16 = mybir.dt.bfloat16
    AF = mybir.ActivationFunctionType

    nc = bacc.Bacc(
        "TRN2",
        target_bir_lowering=False,
        debug=False,
        enable_asserts=False,
        num_devices=N_CORES,
    )
    xx_d = nc.dram_tensor("xx", [128, 2 * FB], bf16, kind="ExternalInput")
    out_d = nc.dram_tensor("out", [128, 48], f32, kind="ExternalOutput")

    with tile.TileContext(nc) as tc:
        with tc.tile_pool(name="pers", bufs=1) as pers:
            acc = pers.tile([128, 48], f32, tag="acc")
            sw0 = pers.tile([128, 9680], bf16, tag="sw0")
            sw1 = pers.tile([128, 9680], bf16, tag="sw1")
            dw0 = pers.tile([128, 9680], bf16, tag="dw0")
            dw1 = pers.tile([128, 9680], bf16, tag="dw1")
            swf = [sw0, sw1]
            dwf = [dw0, dw1]

            # ---- w-stage over the full volume, chunked by DMA arrival
            with tc.tile_pool(name="xp", bufs=1) as xp:
                xw = xp.tile([128, 2 * FB], bf16, tag="xw")
                U = xp.tile([128, 132 * 21], bf16, tag="u")
                for p0, p1 in CHUNKS:
                    for i in range(2):
                        nc.sync.dma_start(
                            out=xw[:, i * FB + p0 * PL:i * FB + p1 * PL],
                            in_=xx_d[:, i * FB + p0 * PL:i * FB + p1 * PL])
                for p0, p1 in CHUNKS:
                    n = (p1 - p0) * BB   # rows in chunk
                    for i in range(2):
                        xr = xw[:, i * FB + p0 * PL:i * FB + p1 * PL] \
                            .rearrange("p (r w) -> p r w", w=BB)
                        u21 = U[:, 0:n * 21].rearrange("p (r w) -> p r w",
                                                       w=21)
                        nc.vector.tensor_add(u21[:, :, :],
                                             xr[:, :, 0:21], xr[:, :, 1:22])
                        swd = swf[i][:, p0 * 440:p1 * 440] \
                            .rearrange("p (r w) -> p r w", w=BC)
                        nc.vector.tensor_add(swd[:, :, :],
                                             u21[:, :, 0:20], u21[:, :, 1:21])
                        dwd = dwf[i][:, p0 * 440:p1 * 440] \
                            .rearrange("p (r w) -> p r w", w=BC)
                        nc.vector.tensor_sub(dwd[:, :, :],
                                             xr[:, :, 2:22], xr[:, :, 0:20])

            # ---- slab phase: h/d stages + pointwise, flat dense ops
            with tc.tile_pool(name="work", bufs=1) as work, \
                 tc.tile_pool(name="fld", bufs=1) as fld, \
                 tc.tile_pool(name="late", bufs=1) as late:
                sqa = late.tile([128, 2 * HF], bf16, tag="sq0")
                sqb = late.tile([128, 2 * HF], bf16, tag="sq1")
                sqc = late.tile([128, 2 * HF], bf16, tag="sq2")
                sq = [sqa, sqb, sqc]
                stile = late.tile([128, 2 * HF], bf16, tag="s")
                M12 = late.tile([128, HF], bf16, tag="m12")
                DOT = late.tile([128, HF], bf16, tag="dot")
                Q = late.tile([128, HF], bf16, tag="q")
                RT = late.tile([128, HF], bf16, tag="r")
                SCR = late.tile([128, HF], bf16, tag="scr")
                CT = SCR   # c-mult output reuses scr (disjoint lifetimes)
                LNQ = late.tile([128, HF], f32, tag="lnq")

                def pv(t, off, np_, a, b, w=440):
                    """Per-plane valid view: [[w, np_],[1, b-a]]."""
                    return t[:, off:off + np_ * w] \
                        .rearrange("p (r w) -> p r w", w=w)[:, :, a:b]

                def conv_hd(s, mid=None):
                    """h/d stages + squares for one slab (both inputs).
                    Per-plane views skip the cross-plane junk h-rows
                    entirely: no junk computed, read, or corrected."""
                    o = 2200 * s
                    ga = fld.tile([128, 2 * HF], bf16, tag="g0")
                    gb = fld.tile([128, 2 * HF], bf16, tag="g1")
                    gc = fld.tile([128, 2 * HF], bf16, tag="g2")
                    gt = [ga, gb, gc]
                    for i in range(2):
                        sw, dw = swf[i], dwf[i]
                        W0 = work.tile([128, 3060], bf16, tag="W0")
                        W1 = work.tile([128, 3040], bf16, tag="W1")
                        W2 = work.tile([128, 3060], bf16, tag="W2")
                        W3 = work.tile([128, 3040], bf16, tag="W3")
                        W4 = work.tile([128, 3040], bf16, tag="W4")
                        uh1, dhsw, uh2, shsw, shdw = W0, W1, W2, W3, W4
                        nc.vector.tensor_add(pv(uh1, 0, 7, 0, 420, 420),
                                             pv(sw, o, 7, 0, 420),
                                             pv(sw, o, 7, 20, 440))
                        nc.vector.tensor_sub(pv(dhsw, 0, 7, 0, 400, 400),
                                             pv(sw, o, 7, 40, 440),
                                             pv(sw, o, 7, 0, 400))
                        nc.vector.tensor_add(pv(uh2, 0, 7, 0, 420, 420),
                                             pv(dw, o, 7, 0, 420),
                                             pv(dw, o, 7, 20, 440))
                        if i == 1 and mid is not None:
                            # fire the previous slab's c-mult here: ~3 ops
                            # into input 1, ScalarE's r is ready by now
                            mid()
                        nc.vector.tensor_add(pv(shsw, 0, 7, 0, 400, 400),
                                             pv(uh1, 0, 7, 0, 400, 420),
                                             pv(uh1, 0, 7, 20, 420, 420))
                        nc.vector.tensor_add(pv(shdw, 0, 7, 0, 400, 400),
                                             pv(uh2, 0, 7, 0, 400, 420),
                                             pv(uh2, 0, 7, 20, 420, 420))
                        ud1 = work.tile([128, 2600], bf16, tag="U1")
                        ud2 = work.tile([128, 2600], bf16, tag="U2")
                        nc.vector.tensor_add(ud1[:, 0:2400],
                                             shdw[:, 0:2400],
                                             shdw[:, 400:2800])
                        nc.vector.tensor_add(ud2[:, 0:2400],
                                             dhsw[:, 0:2400],
                                             dhsw[:, 400:2800])
                        h0 = i * HF
                        nc.vector.tensor_add(gt[0][:, h0:h0 + 2000],
                                             ud1[:, 0:2000], ud1[:, 400:2400])
                        nc.vector.tensor_add(gt[1][:, h0:h0 + 2000],
                                             ud2[:, 0:2000], ud2[:, 400:2400])
                        nc.vector.tensor_sub(gt[2][:, h0:h0 + 2000],
                                             shsw[:, 800:2800], shsw[:, 0:2000])
                        for k in range(3):
                            nc.scalar.activation(
                                sq[k][:, h0:h0 + 2000],
                                gt[k][:, h0:h0 + 2000], AF.Square,
                                accum_out=acc[:, 8 * s + 2 * k + i:
                                              8 * s + 2 * k + i + 1])
                    return gt

                def pw_a_v(F, p0=0, np_=5):
                    """V: dot products, s sums, q on planes [p0, p0+np_)."""
                    a, b = p0 * 400, (p0 + np_) * 400
                    MT = (Q, RT, SCR)
                    for m, g in zip(MT, F):
                        nc.vector.tensor_mul(m[:, a:b], g[:, a:b],
                                             g[:, HF + a:HF + b])
                    nc.vector.tensor_add(M12[:, a:b],
                                         MT[0][:, a:b], MT[1][:, a:b])
                    nc.vector.tensor_add(DOT[:, a:b],
                                         M12[:, a:b], MT[2][:, a:b])
                    for i in range(2):
                        h0 = i * HF
                        nc.vector.tensor_add(M12[:, a:b],
                                             sq[0][:, h0 + a:h0 + b],
                                             sq[1][:, h0 + a:h0 + b])
                        nc.vector.tensor_add(stile[:, h0 + a:h0 + b],
                                             M12[:, a:b],
                                             sq[2][:, h0 + a:h0 + b])
                    nc.vector.tensor_mul(Q[:, a:b], stile[:, a:b],
                                         stile[:, HF + a:HF + b])

                def pw_a_ln(p0=0, np_=5):
                    a, b = p0 * 400, (p0 + np_) * 400
                    nc.scalar.activation(LNQ[:, a:b], Q[:, a:b], AF.Ln)

                def pw_a_exp(cb, p0=0, np_=5):
                    a, b = p0 * 400, (p0 + np_) * 400
                    # r first: the downstream c-mult waits only on r
                    nc.scalar.activation(RT[:, a:b], LNQ[:, a:b],
                                         AF.Exp, scale=-0.5)
                    nc.scalar.activation(SCR[:, a:b], LNQ[:, a:b],
                                         AF.Exp, scale=0.5,
                                         accum_out=acc[:, 32 + cb:33 + cb])

                def pw_b_v(p0=0, np_=5):
                    a, b = p0 * 400, (p0 + np_) * 400
                    nc.vector.tensor_mul(CT[:, a:b], DOT[:, a:b], RT[:, a:b])

                def pw_b_s(cb, p0=0, np_=5):
                    a, b = p0 * 400, (p0 + np_) * 400
                    nc.scalar.activation(M12[:, a:b], CT[:, a:b], AF.Copy,
                                         accum_out=acc[:, 40 + cb:41 + cb])

                def pw_b_vred(cb, p0, np_):
                    a, b = p0 * 400, (p0 + np_) * 400
                    nc.vector.tensor_mul(CT[:, a:b], DOT[:, a:b], RT[:, a:b])
                    nc.vector.tensor_reduce(acc[:, 40 + cb:41 + cb],
                                            CT[:, a:b],
                                            axis=mybir.AxisListType.X,
                                            op=mybir.AluOpType.add)

                F0 = conv_hd(0)
                pw_a_v(F0)
                pw_a_ln()
                pw_a_exp(0)
                F1 = conv_hd(1, mid=lambda: (pw_b_v(), pw_b_s(0)))
                pw_a_v(F1)
                pw_a_ln()
                pw_a_exp(1)
                F2 = conv_hd(2, mid=lambda: (pw_b_v(), pw_b_s(1)))
                pw_a_v(F2)
                pw_a_ln()
                pw_a_exp(2)
                F3 = conv_hd(3, mid=lambda: (pw_b_v(), pw_b_s(2)))
                pw_a_v(F3, 0, 3)
                pw_a_v(F3, 3, 2)
                pw_a_ln(0, 3)
                pw_a_ln(3, 2)
                pw_a_exp(3, 0, 3)
                pw_b_vred(3, 0, 3)
                pw_a_exp(4, 3, 2)
                pw_b_vred(4, 3, 2)

                nc.sync.dma_start(out=out_d[:], in_=acc[:])

    nc.compile()
    return nc


def _shard_inputs(pred, target):
    bf = ml_dtypes.bfloat16
    starts = np.arange(0, 160, BC)

    blocked = {}
    for name, x in (("pred", pred), ("targ", target)):
        per_b = []
        for b in range(2):
            gp = np.zeros((162, 162, 162), np.float32)
            gp[1:161, 1:161, 1:161] = x[b, 0]
            swv = np.lib.stride_tricks.sliding_window_view(gp, (BB, BB, BB))
            per_b.append(swv)
        blocked[name] = per_b

    in_maps = []
    for core in range(N_CORES):
        b, q = divmod(core, 4)
        xx = np.empty((128, 2 * FB), bf)
        for i, name in enumerate(("pred", "targ")):
            swv = blocked[name][b]
            blk = swv[np.ix_([40 * q, 40 * q + BC], starts, starts)]
            xx[:, i * FB:(i + 1) * FB] = blk.reshape(128, FB).astype(bf)
        in_maps.append({"xx": xx})
    return in_maps


def run(pred, target, trace=False):
    from concourse.bass_utils import run_bass_kernel_spmd

    pred = np.asarray(pred, dtype=np.float32)
    target = np.asarray(target, dtype=np.float32)
    assert pred.shape == (2, 1, 160, 160, 160)

    if "nc" not in _cache:
        _cache["nc"] = _build()
    nc = _cache["nc"]

    in_maps = _shard_inputs(pred, target)
    res = None
    for attempt in range(3):
        try:
            res = run_bass_kernel_spmd(
                nc, in_maps, core_ids=list(range(N_CORES)), trace=trace)
            break
        except Exception:
            if attempt == 2:
                raise
            import time as _time
            _time.sleep(5)

    sq_sum = 0.0
    sqrt_sum = 0.0
    c_sum = 0.0
    for core_out in res.results:
        o = np.asarray(core_out["out"], np.float64)
        for s in range(NSLAB):
            sq_sum += o[:, 8 * s:8 * s + 6].sum()
        sqrt_sum += o[:, 32:37].sum()
        c_sum += o[:, 40:45].sum()

    mag_sum = sq_sum - 2.0 * sqrt_sum
    loss = WEIGHT * (mag_sum / NVOX + 1.0 - c_sum / NVOX)
    return np.float32(loss), res.exec_time_ns


def kernel(pred, target):
    loss, _ = run(pred, target, trace=False)
    return loss


# revision 7
# speedup vs baseline: 1.3019x; 1.0082x over previous
"""Trainium2 distributed kernel for AnatomicalConsistencyLoss, v3.

Like v2 (see kernel_v2.py docstring for the engine-assignment rationale:
VectorE does all tensor_tensor work in the 2x bf16 mode, ScalarE all
unaries + accum reductions, other engines measured counterproductive),
plus:

- The w-stage (S_w / D_w) runs ONCE over the full 22-plane input volume
  instead of per-slab (saves the 2-plane slab halo reprocessing), tiled
  by DMA chunk for overlap with the input transfer.
- sw/dw and everything downstream are stored with dense 20-wide rows,
  so all h/d-stage and pointwise ops are flat 1-D contiguous slices
  (junk h-rows remain, zeroed/one'd in the field tiles and exactly
  subtracted host-side).
- The input tile and u-scratch live in a pool closed after the w-stage
  so the slab-phase tiles reuse their SBUF.
"""

import sys

import numpy as np

sys.path.insert(0, "/opt/trn_rl_repo")

import ml_dtypes

N_CORES = 8
BC = 20
BB = 22
PL = BB * BB     # 484
FB = BB ** 3     # 10648
NVOX = 2 * 160 * 160 * 160
WEIGHT = 0.2

SD = 5
NSLAB = 4
HF = SD * BC * BB    # 2200: dense field half size
NR = SD * BB         # 110 rows per half
JUNK_PER_SLAB = 10 * BC

# DMA/w-stage chunks in planes
CHUNKS = [(0, 3), (3, 8), (8, 13), (13, 17), (17, 22)]

_cache = {}


def _build():
    import concourse.bacc as bacc
    import concourse.tile as tile
    from concourse import mybir

    f32 = mybir.dt.float32
    bf16 = mybir.dt.bfloat16
    AF = mybir.ActivationFunctionType

    nc = bacc.Bacc(
        "TRN2",
        target_bir_lowering=False,
        debug=False,
        enable_asserts=False,
        num_devices=N_CORES,
    )
    xx_d = nc.dram_tensor("xx", [128, 2 * FB], bf16, kind="ExternalInput")
    out_d = nc.dram_tensor("out", [128, 48], f32, kind="ExternalOutput")

    with tile.TileContext(nc) as tc:
        with tc.tile_pool(name="pers", bufs=1) as pers:
            acc = pers.tile([128, 48], f32, tag="acc")
            sw0 = pers.tile([128, 9680], bf16, tag="sw0")
            sw1 = pers.tile([128, 9680], bf16, tag="sw1")
            dw0 = pers.tile([128, 9680], bf16, tag="dw0")
            dw1 = pers.tile([128, 9680], bf16, tag="dw1")
            swf = [sw0, sw1]
            dwf = [dw0, dw1]

            # ---- w-stage over the full volume, chunked by DMA arrival
            with tc.tile_pool(name="xp", bufs=1) as xp:
                xw = xp.tile([128, 2 * FB], bf16, tag="xw")
                U = xp.tile([128, 132 * 21], bf16, tag="u")
                for p0, p1 in CHUNKS:
                    for i in range(2):
                        nc.sync.dma_start(
                            out=xw[:, i * FB + p0 * PL:i * FB + p1 * PL],
                            in_=xx_d[:, i * FB + p0 * PL:i * FB + p1 * PL])
                for p0, p1 in CHUNKS:
                    n = (p1 - p0) * BB   # rows in chunk
                    for i in range(2):
                        xr = xw[:, i * FB + p0 * PL:i * FB + p1 * PL] \
                            .rearrange("p (r w) -> p r w", w=BB)
                        u21 = U[:, 0:n * 21].rearrange("p (r w) -> p r w",
                                                       w=21)
                        nc.vector.tensor_add(u21[:, :, :],
                                             xr[:, :, 0:21], xr[:, :, 1:22])
                        swd = swf[i][:, p0 * 440:p1 * 440] \
                            .rearrange("p (r w) -> p r w", w=BC)
                        nc.vector.tensor_add(swd[:, :, :],
                                             u21[:, :, 0:20], u21[:, :, 1:21])
                        dwd = dwf[i][:, p0 * 440:p1 * 440] \
                            .rearrange("p (r w) -> p r w", w=BC)
                        nc.vector.tensor_sub(dwd[:, :, :],
                                             xr[:, :, 2:22], xr[:, :, 0:20])

            # ---- slab phase: h/d stages + pointwise, flat dense ops
            with tc.tile_pool(name="work", bufs=1) as work, \
                 tc.tile_pool(name="fld", bufs=1) as fld, \
                 tc.tile_pool(name="late", bufs=1) as late:
                sqa = late.tile([128, 2 * HF], bf16, tag="sq0")
                sqb = late.tile([128, 2 * HF], bf16, tag="sq1")
                sqc = late.tile([128, 2 * HF], bf16, tag="sq2")
                sq = [sqa, sqb, sqc]
                stile = late.tile([128, 2 * HF], bf16, tag="s")
                M12 = late.tile([128, HF], bf16, tag="m12")
                DOT = None  # dot lives in Q (m1's slot, dead after m12)
                Q = late.tile([128, HF], bf16, tag="q")
                QQ = late.tile([128, HF], bf16, tag="qq")
                RT = late.tile([128, HF], bf16, tag="r")
                SCR = late.tile([128, HF], bf16, tag="scr")
                CT = SCR   # c-mult output reuses scr (disjoint lifetimes)
                LNQ = late.tile([128, HF], f32, tag="lnq")

                def pv(t, off, np_, a, b, w=440):
                    """Per-plane valid view: [[w, np_],[1, b-a]]."""
                    return t[:, off:off + np_ * w] \
                        .rearrange("p (r w) -> p r w", w=w)[:, :, a:b]

                def conv_hd(s, mid=None):
                    """h/d stages + squares for one slab (both inputs).
                    Per-plane views skip the cross-plane junk h-rows
                    entirely: no junk computed, read, or corrected."""
                    o = 2200 * s
                    ga = fld.tile([128, 2 * HF], bf16, tag="g0")
                    gb = fld.tile([128, 2 * HF], bf16, tag="g1")
                    gc = fld.tile([128, 2 * HF], bf16, tag="g2")
                    gt = [ga, gb, gc]
                    for i in range(2):
                        sw, dw = swf[i], dwf[i]
                        W0 = work.tile([128, 3060], bf16, tag="W0")
                        W1 = work.tile([128, 3040], bf16, tag="W1")
                        W2 = work.tile([128, 3060], bf16, tag="W2")
                        W3 = work.tile([128, 3040], bf16, tag="W3")
                        W4 = work.tile([128, 3040], bf16, tag="W4")
                        uh1, dhsw, uh2, shsw, shdw = W0, W1, W2, W3, W4
                        nc.vector.tensor_add(pv(uh1, 0, 7, 0, 420, 420),
                                             pv(sw, o, 7, 0, 420),
                                             pv(sw, o, 7, 20, 440))
                        nc.vector.tensor_sub(pv(dhsw, 0, 7, 0, 400, 400),
                                             pv(sw, o, 7, 40, 440),
                                             pv(sw, o, 7, 0, 400))
                        nc.vector.tensor_add(pv(uh2, 0, 7, 0, 420, 420),
                                             pv(dw, o, 7, 0, 420),
                                             pv(dw, o, 7, 20, 440))
                        if i == 1 and mid is not None:
                            # fire the previous slab's c-mult here: ~3 ops
                            # into input 1, ScalarE's r is ready by now
                            mid()
                        nc.vector.tensor_add(pv(shsw, 0, 7, 0, 400, 400),
                                             pv(uh1, 0, 7, 0, 400, 420),
                                             pv(uh1, 0, 7, 20, 420, 420))
                        nc.vector.tensor_add(pv(shdw, 0, 7, 0, 400, 400),
                                             pv(uh2, 0, 7, 0, 400, 420),
                                             pv(uh2, 0, 7, 20, 420, 420))
                        ud1 = work.tile([128, 2600], bf16, tag="U1")
                        ud2 = work.tile([128, 2600], bf16, tag="U2")
                        nc.vector.tensor_add(ud1[:, 0:2400],
                                             shdw[:, 0:2400],
                                             shdw[:, 400:2800])
                        nc.vector.tensor_add(ud2[:, 0:2400],
                                             dhsw[:, 0:2400],
                                             dhsw[:, 400:2800])
                        h0 = i * HF

                        def square(k):
                            nc.scalar.activation(
                                sq[k][:, h0:h0 + 2000],
                                gt[k][:, h0:h0 + 2000], AF.Square,
                                accum_out=acc[:, 8 * s + 2 * k + i:
                                              8 * s + 2 * k + i + 1])
                        # each square emitted right after its field write
                        # so ScalarE starts as early as possible
                        nc.vector.tensor_add(gt[0][:, h0:h0 + 2000],
                                             ud1[:, 0:2000], ud1[:, 400:2400])
                        square(0)
                        nc.vector.tensor_add(gt[1][:, h0:h0 + 2000],
                                             ud2[:, 0:2000], ud2[:, 400:2400])
                        square(1)
                        nc.vector.tensor_sub(gt[2][:, h0:h0 + 2000],
                                             shsw[:, 800:2800], shsw[:, 0:2000])
                        square(2)
                    return gt

                def pw_a_v(F, p0=0, np_=5):
                    """V: products, s sums, q early (frees ScalarE's ln
                    chain), then m12/dot (not needed until next conv)."""
                    a, b = p0 * 400, (p0 + np_) * 400
                    MT = (Q, RT, SCR)
                    for m, g in zip(MT, F):
                        nc.vector.tensor_mul(m[:, a:b], g[:, a:b],
                                             g[:, HF + a:HF + b])
                    for i in range(2):
                        h0 = i * HF
                        nc.vector.tensor_add(M12[:, a:b],
                                             sq[0][:, h0 + a:h0 + b],
                                             sq[1][:, h0 + a:h0 + b])
                        nc.vector.tensor_add(stile[:, h0 + a:h0 + b],
                                             M12[:, a:b],
                                             sq[2][:, h0 + a:h0 + b])
                    nc.vector.tensor_mul(QQ[:, a:b], stile[:, a:b],
                                         stile[:, HF + a:HF + b])
                    nc.vector.tensor_add(M12[:, a:b],
                                         MT[0][:, a:b], MT[1][:, a:b])
                    nc.vector.tensor_add(Q[:, a:b],
                                         M12[:, a:b], MT[2][:, a:b])

                def pw_a_ln(p0=0, np_=5):
                    a, b = p0 * 400, (p0 + np_) * 400
                    nc.scalar.activation(LNQ[:, a:b], QQ[:, a:b], AF.Ln)

                def pw_a_exp(cb, p0=0, np_=5):
                    a, b = p0 * 400, (p0 + np_) * 400
                    # r first: the downstream c-mult waits only on r
                    nc.scalar.activation(RT[:, a:b], LNQ[:, a:b],
                                         AF.Exp, scale=-0.5)
                    nc.scalar.activation(SCR[:, a:b], LNQ[:, a:b],
                                         AF.Exp, scale=0.5,
                                         accum_out=acc[:, 32 + cb:33 + cb])

                def pw_b_v(p0=0, np_=5):
                    a, b = p0 * 400, (p0 + np_) * 400
                    nc.vector.tensor_mul(CT[:, a:b], Q[:, a:b], RT[:, a:b])

                def pw_b_s(cb, p0=0, np_=5):
                    a, b = p0 * 400, (p0 + np_) * 400
                    nc.scalar.activation(M12[:, a:b], CT[:, a:b], AF.Copy,
                                         accum_out=acc[:, 40 + cb:41 + cb])

                def pw_b_vred(cb, p0, np_):
                    a, b = p0 * 400, (p0 + np_) * 400
                    nc.vector.tensor_mul(CT[:, a:b], Q[:, a:b], RT[:, a:b])
                    nc.vector.tensor_reduce(acc[:, 40 + cb:41 + cb],
                                            CT[:, a:b],
                                            axis=mybir.AxisListType.X,
                                            op=mybir.AluOpType.add)

                F0 = conv_hd(0)
                pw_a_v(F0)
                pw_a_ln()
                pw_a_exp(0)
                F1 = conv_hd(1, mid=lambda: (pw_b_v(), pw_b_s(0)))
                pw_a_v(F1)
                pw_a_ln()
                pw_a_exp(1)
                F2 = conv_hd(2, mid=lambda: (pw_b_v(), pw_b_s(1)))
                pw_a_v(F2)
                pw_a_ln()
                pw_a_exp(2)
                F3 = conv_hd(3, mid=lambda: (pw_b_v(), pw_b_s(2)))
                pw_a_v(F3, 0, 3)
                pw_a_v(F3, 3, 2)
                pw_a_ln(0, 3)
                pw_a_ln(3, 2)
                pw_a_exp(3, 0, 3)
                pw_b_vred(3, 0, 3)
                pw_a_exp(4, 3, 2)
                pw_b_vred(4, 3, 2)

                nc.sync.dma_start(out=out_d[:], in_=acc[:])

    nc.compile()
    return nc


def _shard_inputs(pred, target):
    bf = ml_dtypes.bfloat16
    starts = np.arange(0, 160, BC)

    blocked = {}
    for name, x in (("pred", pred), ("targ", target)):
        per_b = []
        for b in range(2):
            gp = np.zeros((162, 162, 162), np.float32)
            gp[1:161, 1:161, 1:161] = x[b, 0]
            swv = np.lib.stride_tricks.sliding_window_view(gp, (BB, BB, BB))
            per_b.append(swv)
        blocked[name] = per_b

    in_maps = []
    for core in range(N_CORES):
        b, q = divmod(core, 4)
        xx = np.empty((128, 2 * FB), bf)
        for i, name in enumerate(("pred", "targ")):
            swv = blocked[name][b]
            blk = swv[np.ix_([40 * q, 40 * q + BC], starts, starts)]
            xx[:, i * FB:(i + 1) * FB] = blk.reshape(128, FB).astype(bf)
        in_maps.append({"xx": xx})
    return in_maps


def run(pred, target, trace=False):
    from concourse.bass_utils import run_bass_kernel_spmd

    pred = np.asarray(pred, dtype=np.float32)
    target = np.asarray(target, dtype=np.float32)
    assert pred.shape == (2, 1, 160, 160, 160)

    if "nc" not in _cache:
        _cache["nc"] = _build()
    nc = _cache["nc"]

    in_maps = _shard_inputs(pred, target)
    res = None
    for attempt in range(3):
        try:
            res = run_bass_kernel_spmd(
                nc, in_maps, core_ids=list(range(N_CORES)), trace=trace)
            break
        except Exception:
            if attempt == 2:
                raise
            import time as _time
            _time.sleep(5)

    sq_sum = 0.0
    sqrt_sum = 0.0
    c_sum = 0.0
    for core_out in res.results:
        o = np.asarray(core_out["out"], np.float64)
        for s in range(NSLAB):
            sq_sum += o[:, 8 * s:8 * s + 6].sum()
        sqrt_sum += o[:, 32:37].sum()
        c_sum += o[:, 40:45].sum()

    mag_sum = sq_sum - 2.0 * sqrt_sum
    loss = WEIGHT * (mag_sum / NVOX + 1.0 - c_sum / NVOX)
    return np.float32(loss), res.exec_time_ns


def kernel(pred, target):
    loss, _ = run(pred, target, trace=False)
    return loss


# revision 8
# speedup vs baseline: 1.3121x; 1.0078x over previous
"""Trainium2 distributed kernel for AnatomicalConsistencyLoss, v3.

Like v2 (see kernel_v2.py docstring for the engine-assignment rationale:
VectorE does all tensor_tensor work in the 2x bf16 mode, ScalarE all
unaries + accum reductions, other engines measured counterproductive),
plus:

- The w-stage (S_w / D_w) runs ONCE over the full 22-plane input volume
  instead of per-slab (saves the 2-plane slab halo reprocessing), tiled
  by DMA chunk for overlap with the input transfer.
- sw/dw and everything downstream are stored with dense 20-wide rows,
  so all h/d-stage and pointwise ops are flat 1-D contiguous slices
  (junk h-rows remain, zeroed/one'd in the field tiles and exactly
  subtracted host-side).
- The input tile and u-scratch live in a pool closed after the w-stage
  so the slab-phase tiles reuse their SBUF.
"""

import sys

import numpy as np

sys.path.insert(0, "/opt/trn_rl_repo")

import ml_dtypes

N_CORES = 8
BC = 20
BB = 22
PL = BB * BB     # 484
FB = BB ** 3     # 10648
NVOX = 2 * 160 * 160 * 160
WEIGHT = 0.2

SD = 5
NSLAB = 4
HF = SD * 400        # 2000: fully dense field half size
NR = SD * BB         # 110 rows per half
JUNK_PER_SLAB = 10 * BC

# DMA/w-stage chunks in planes
CHUNKS = [(0, 3), (3, 8), (8, 13), (13, 17), (17, 22)]

_cache = {}


def _build():
    import concourse.bacc as bacc
    import concourse.tile as tile
    from concourse import mybir

    f32 = mybir.dt.float32
    bf16 = mybir.dt.bfloat16
    AF = mybir.ActivationFunctionType

    nc = bacc.Bacc(
        "TRN2",
        target_bir_lowering=False,
        debug=False,
        enable_asserts=False,
        num_devices=N_CORES,
    )
    xx_d = nc.dram_tensor("xx", [128, 2 * FB], bf16, kind="ExternalInput")
    out_d = nc.dram_tensor("out", [128, 48], f32, kind="ExternalOutput")

    with tile.TileContext(nc) as tc:
        with tc.tile_pool(name="pers", bufs=1) as pers:
            acc = pers.tile([128, 48], f32, tag="acc")
            sw0 = pers.tile([128, 9680], bf16, tag="sw0")
            sw1 = pers.tile([128, 9680], bf16, tag="sw1")
            dw0 = pers.tile([128, 9680], bf16, tag="dw0")
            dw1 = pers.tile([128, 9680], bf16, tag="dw1")
            swf = [sw0, sw1]
            dwf = [dw0, dw1]

            # ---- w-stage over the full volume, chunked by DMA arrival
            with tc.tile_pool(name="xp", bufs=1) as xp:
                xw = xp.tile([128, 2 * FB], bf16, tag="xw")
                U = xp.tile([128, 132 * 21], bf16, tag="u")
                for p0, p1 in CHUNKS:
                    for i in range(2):
                        nc.sync.dma_start(
                            out=xw[:, i * FB + p0 * PL:i * FB + p1 * PL],
                            in_=xx_d[:, i * FB + p0 * PL:i * FB + p1 * PL])
                for p0, p1 in CHUNKS:
                    n = (p1 - p0) * BB   # rows in chunk
                    for i in range(2):
                        xr = xw[:, i * FB + p0 * PL:i * FB + p1 * PL] \
                            .rearrange("p (r w) -> p r w", w=BB)
                        u21 = U[:, 0:n * 21].rearrange("p (r w) -> p r w",
                                                       w=21)
                        nc.vector.tensor_add(u21[:, :, :],
                                             xr[:, :, 0:21], xr[:, :, 1:22])
                        swd = swf[i][:, p0 * 440:p1 * 440] \
                            .rearrange("p (r w) -> p r w", w=BC)
                        nc.vector.tensor_add(swd[:, :, :],
                                             u21[:, :, 0:20], u21[:, :, 1:21])
                        dwd = dwf[i][:, p0 * 440:p1 * 440] \
                            .rearrange("p (r w) -> p r w", w=BC)
                        nc.vector.tensor_sub(dwd[:, :, :],
                                             xr[:, :, 2:22], xr[:, :, 0:20])

            # ---- slab phase: h/d stages + pointwise, flat dense ops
            with tc.tile_pool(name="work", bufs=1) as work, \
                 tc.tile_pool(name="fld", bufs=1) as fld, \
                 tc.tile_pool(name="late", bufs=1) as late:
                sqa = late.tile([128, 2 * HF], bf16, tag="sq0")
                sqb = late.tile([128, 2 * HF], bf16, tag="sq1")
                sqc = late.tile([128, 2 * HF], bf16, tag="sq2")
                sq = [sqa, sqb, sqc]
                stile = late.tile([128, 2 * HF], bf16, tag="s")
                M12 = late.tile([128, 2 * HF], bf16, tag="m12")
                DOT = None  # dot lives in Q (m1's slot, dead after m12)
                Q = late.tile([128, HF], bf16, tag="q")
                QQ = late.tile([128, HF], bf16, tag="qq")
                RT = late.tile([128, HF], bf16, tag="r")
                SCR = late.tile([128, HF], bf16, tag="scr")
                CT = SCR   # c-mult output reuses scr (disjoint lifetimes)
                LNQ = late.tile([128, HF], f32, tag="lnq")

                def pv(t, off, np_, a, b, w=440):
                    """Per-plane valid view: [[w, np_],[1, b-a]]."""
                    return t[:, off:off + np_ * w] \
                        .rearrange("p (r w) -> p r w", w=w)[:, :, a:b]

                def conv_hd(s, mid=None):
                    """h/d stages + squares for one slab (both inputs).
                    Per-plane views skip the cross-plane junk h-rows
                    entirely: no junk computed, read, or corrected."""
                    o = 2200 * s
                    ga = fld.tile([128, 2 * HF], bf16, tag="g0")
                    gb = fld.tile([128, 2 * HF], bf16, tag="g1")
                    gc = fld.tile([128, 2 * HF], bf16, tag="g2")
                    gt = [ga, gb, gc]
                    for i in range(2):
                        sw, dw = swf[i], dwf[i]
                        W0 = work.tile([128, 3060], bf16, tag="W0")
                        W1 = work.tile([128, 3040], bf16, tag="W1")
                        W2 = work.tile([128, 3060], bf16, tag="W2")
                        W3 = work.tile([128, 3040], bf16, tag="W3")
                        W4 = work.tile([128, 3040], bf16, tag="W4")
                        uh1, dhsw, uh2, shsw, shdw = W0, W1, W2, W3, W4
                        nc.vector.tensor_add(pv(uh1, 0, 7, 0, 420, 420),
                                             pv(sw, o, 7, 0, 420),
                                             pv(sw, o, 7, 20, 440))
                        nc.vector.tensor_sub(pv(dhsw, 0, 7, 0, 400, 400),
                                             pv(sw, o, 7, 40, 440),
                                             pv(sw, o, 7, 0, 400))
                        nc.vector.tensor_add(pv(uh2, 0, 7, 0, 420, 420),
                                             pv(dw, o, 7, 0, 420),
                                             pv(dw, o, 7, 20, 440))
                        if i == 1 and mid is not None:
                            # fire the previous slab's c-mult here: ~3 ops
                            # into input 1, ScalarE's r is ready by now
                            mid()
                        nc.vector.tensor_add(pv(shsw, 0, 7, 0, 400, 400),
                                             pv(uh1, 0, 7, 0, 400, 420),
                                             pv(uh1, 0, 7, 20, 420, 420))
                        nc.vector.tensor_add(pv(shdw, 0, 7, 0, 400, 400),
                                             pv(uh2, 0, 7, 0, 400, 420),
                                             pv(uh2, 0, 7, 20, 420, 420))
                        ud1 = work.tile([128, 2600], bf16, tag="U1")
                        ud2 = work.tile([128, 2600], bf16, tag="U2")
                        nc.vector.tensor_add(ud1[:, 0:2400],
                                             shdw[:, 0:2400],
                                             shdw[:, 400:2800])
                        nc.vector.tensor_add(ud2[:, 0:2400],
                                             dhsw[:, 0:2400],
                                             dhsw[:, 400:2800])
                        h0 = i * HF

                        def square(k):
                            nc.scalar.activation(
                                sq[k][:, h0:h0 + 2000],
                                gt[k][:, h0:h0 + 2000], AF.Square,
                                accum_out=acc[:, 8 * s + 2 * k + i:
                                              8 * s + 2 * k + i + 1])
                        # each square emitted right after its field write
                        # so ScalarE starts as early as possible
                        nc.vector.tensor_add(gt[0][:, h0:h0 + 2000],
                                             ud1[:, 0:2000], ud1[:, 400:2400])
                        square(0)
                        nc.vector.tensor_add(gt[1][:, h0:h0 + 2000],
                                             ud2[:, 0:2000], ud2[:, 400:2400])
                        square(1)
                        nc.vector.tensor_sub(gt[2][:, h0:h0 + 2000],
                                             shsw[:, 800:2800], shsw[:, 0:2000])
                        square(2)
                    return gt

                def pw_a_v(F, p0=0, np_=5):
                    """V: products, s sums, q early (frees ScalarE's ln
                    chain), then m12/dot (not needed until next conv)."""
                    a, b = p0 * 400, (p0 + np_) * 400
                    MT = (Q, RT, SCR)
                    for m, g in zip(MT, F):
                        nc.vector.tensor_mul(m[:, a:b], g[:, a:b],
                                             g[:, HF + a:HF + b])
                    if p0 == 0 and np_ == 5:
                        # halves are adjacent (dense HF): one 4000-elem
                        # op per stage instead of two
                        nc.vector.tensor_add(M12[:, 0:2 * HF],
                                             sq[0][:, 0:2 * HF],
                                             sq[1][:, 0:2 * HF])
                        nc.vector.tensor_add(stile[:, 0:2 * HF],
                                             M12[:, 0:2 * HF],
                                             sq[2][:, 0:2 * HF])
                    else:
                        for i in range(2):
                            h0 = i * HF
                            nc.vector.tensor_add(M12[:, a:b],
                                                 sq[0][:, h0 + a:h0 + b],
                                                 sq[1][:, h0 + a:h0 + b])
                            nc.vector.tensor_add(stile[:, h0 + a:h0 + b],
                                                 M12[:, a:b],
                                                 sq[2][:, h0 + a:h0 + b])
                    nc.vector.tensor_mul(QQ[:, a:b], stile[:, a:b],
                                         stile[:, HF + a:HF + b])
                    nc.vector.tensor_add(M12[:, a:b],
                                         MT[0][:, a:b], MT[1][:, a:b])
                    nc.vector.tensor_add(Q[:, a:b],
                                         M12[:, a:b], MT[2][:, a:b])

                def pw_a_ln(p0=0, np_=5):
                    a, b = p0 * 400, (p0 + np_) * 400
                    nc.scalar.activation(LNQ[:, a:b], QQ[:, a:b], AF.Ln)

                def pw_a_exp(cb, p0=0, np_=5):
                    a, b = p0 * 400, (p0 + np_) * 400
                    # r first: the downstream c-mult waits only on r
                    nc.scalar.activation(RT[:, a:b], LNQ[:, a:b],
                                         AF.Exp, scale=-0.5)
                    nc.scalar.activation(SCR[:, a:b], LNQ[:, a:b],
                                         AF.Exp, scale=0.5,
                                         accum_out=acc[:, 32 + cb:33 + cb])

                def pw_b_v(p0=0, np_=5):
                    a, b = p0 * 400, (p0 + np_) * 400
                    nc.vector.tensor_mul(CT[:, a:b], Q[:, a:b], RT[:, a:b])

                def pw_b_s(cb, p0=0, np_=5):
                    a, b = p0 * 400, (p0 + np_) * 400
                    nc.scalar.activation(M12[:, a:b], CT[:, a:b], AF.Copy,
                                         accum_out=acc[:, 40 + cb:41 + cb])

                def pw_b_vred(cb, p0, np_):
                    a, b = p0 * 400, (p0 + np_) * 400
                    nc.vector.tensor_mul(CT[:, a:b], Q[:, a:b], RT[:, a:b])
                    nc.vector.tensor_reduce(acc[:, 40 + cb:41 + cb],
                                            CT[:, a:b],
                                            axis=mybir.AxisListType.X,
                                            op=mybir.AluOpType.add)

                F0 = conv_hd(0)
                pw_a_v(F0)
                pw_a_ln()
                pw_a_exp(0)
                F1 = conv_hd(1, mid=lambda: (pw_b_v(), pw_b_s(0)))
                pw_a_v(F1)
                pw_a_ln()
                pw_a_exp(1)
                F2 = conv_hd(2, mid=lambda: (pw_b_v(), pw_b_s(1)))
                pw_a_v(F2)
                pw_a_ln()
                pw_a_exp(2)
                F3 = conv_hd(3, mid=lambda: (pw_b_v(), pw_b_s(2)))
                pw_a_v(F3, 0, 3)
                pw_a_v(F3, 3, 2)
                pw_a_ln(0, 3)
                pw_a_ln(3, 2)
                pw_a_exp(3, 0, 3)
                pw_b_vred(3, 0, 3)
                pw_a_exp(4, 3, 2)
                pw_b_vred(4, 3, 2)

                nc.sync.dma_start(out=out_d[:], in_=acc[:])

    nc.compile()
    return nc


def _shard_inputs(pred, target):
    bf = ml_dtypes.bfloat16
    starts = np.arange(0, 160, BC)

    blocked = {}
    for name, x in (("pred", pred), ("targ", target)):
        per_b = []
        for b in range(2):
            gp = np.zeros((162, 162, 162), np.float32)
            gp[1:161, 1:161, 1:161] = x[b, 0]
            swv = np.lib.stride_tricks.sliding_window_view(gp, (BB, BB, BB))
            per_b.append(swv)
        blocked[name] = per_b

    in_maps = []
    for core in range(N_CORES):
        b, q = divmod(core, 4)
        xx = np.empty((128, 2 * FB), bf)
        for i, name in enumerate(("pred", "targ")):
            swv = blocked[name][b]
            blk = swv[np.ix_([40 * q, 40 * q + BC], starts, starts)]
            xx[:, i * FB:(i + 1) * FB] = blk.reshape(128, FB).astype(bf)
        in_maps.append({"xx": xx})
    return in_maps


def run(pred, target, trace=False):
    from concourse.bass_utils import run_bass_kernel_spmd

    pred = np.asarray(pred, dtype=np.float32)
    target = np.asarray(target, dtype=np.float32)
    assert pred.shape == (2, 1, 160, 160, 160)

    if "nc" not in _cache:
        _cache["nc"] = _build()
    nc = _cache["nc"]

    in_maps = _shard_inputs(pred, target)
    res = None
    for attempt in range(3):
        try:
            res = run_bass_kernel_spmd(
                nc, in_maps, core_ids=list(range(N_CORES)), trace=trace)
            break
        except Exception:
            if attempt == 2:
                raise
            import time as _time
            _time.sleep(5)

    sq_sum = 0.0
    sqrt_sum = 0.0
    c_sum = 0.0
    for core_out in res.results:
        o = np.asarray(core_out["out"], np.float64)
        for s in range(NSLAB):
            sq_sum += o[:, 8 * s:8 * s + 6].sum()
        sqrt_sum += o[:, 32:37].sum()
        c_sum += o[:, 40:45].sum()

    mag_sum = sq_sum - 2.0 * sqrt_sum
    loss = WEIGHT * (mag_sum / NVOX + 1.0 - c_sum / NVOX)
    return np.float32(loss), res.exec_time_ns


def kernel(pred, target):
    loss, _ = run(pred, target, trace=False)
    return loss
